# revision 12
# baseline (speedup 1.0000x reference)
"""Trainium2 Bass kernel for a decoder layer (LN->attn->res, LN->attn->FFN->res).

Sharding (8 cores, SPMD single program):
 - Row-parallel phases (LN / out-proj / FFN / residuals): global rows B*S =
   4096 split 512/core: core c owns batch c//4, seq rows [512*(c%4), +512).
 - Attention head-parallel: core c computes 2 global heads {2c, 2c+1} for
   both batches over the full sequence (per-core column-sliced QKV weights).
 - Collectives: one 8-core AllGather of the LN output per attention (the
   cheap direction: 1 MB in / 7 MB received), and one AllToAll of the
   attention outputs (1 MB, ~8x less wire than the AllGather equivalent)
   feeding a local out-projection.
 - Dummy PE matmuls are streamed during both collective windows to keep the
   HAM clock gate at 2.4 GHz (otherwise the post-collective matmuls run at
   1.2 GHz for ~3.4 us).

Layout: activations feature-on-partition ([D, rows]); host pre-transposes x
and post-transposes the output. Matmuls bf16, accumulation fp32, residual fp32.

Masking: reference masks k <= q (strictly-upper attention). Softmax runs
without max-subtraction (scores bounded); both heads' score tiles share one
2-bank PSUM tile so a single Exp covers them; diagonal 128x128 blocks are
masked multiplicatively AFTER exp with a 0/1 upper-strict mask on the vector
engine; fully-masked tiles are skipped via per-tile column prefixes. Softmax
denominators ride in PSUM row 64 via a ones-column appended to V; they are
reshaped to [128,8] through DRAM before the reciprocal (DVE reciprocal is
~6.4ns/elem along the free dim). The all-masked last row (uniform attention
over all 2048 keys) is patched post-normalize with a DVE reduce of V.

LayerNorm stats use a [128,128] ones stationary so every per-row scalar op
runs on all 128 lanes.
"""

import numpy as np
import ml_dtypes

import concourse.bass as bass
import concourse.bacc as bacc
import concourse.tile as tile
import concourse.mybir as mybir
from concourse import bass_utils

B, S, D, H, F = 2, 2048, 1024, 16, 4096
HD = D // H            # 64
NCORES = 8
R = S // 4             # 512 rows per core
KT16 = S // 128        # 16 seq tiles
DT8 = D // 128         # 8 feature tiles of D
FT32 = F // 128        # 32 feature tiles of F
EPS = 1e-5

f32 = mybir.dt.float32
bf16 = mybir.dt.bfloat16
GROUP8 = [list(range(8))]
AF = mybir.ActivationFunctionType
ALU = mybir.AluOpType


def _emit(nc, tc, ctxpools):
    # ---------------- I/O tensors ----------------
    xT = nc.dram_tensor("xT", [D, R], f32, kind="ExternalInput")
    wq_my = nc.dram_tensor("wq_my", [D, 2 * HD], bf16, kind="ExternalInput")
    wk_my = nc.dram_tensor("wk_my", [D, 2 * HD], bf16, kind="ExternalInput")
    wv_my = nc.dram_tensor("wv_my", [D, 2 * HD], bf16, kind="ExternalInput")
    wo = nc.dram_tensor("wo", [D, D], bf16, kind="ExternalInput")
    w1 = nc.dram_tensor("w1", [D, F], bf16, kind="ExternalInput")
    w2 = nc.dram_tensor("w2", [F, D], bf16, kind="ExternalInput")
    bo_c = nc.dram_tensor("bo_c", [D, 1], f32, kind="ExternalInput")
    b1_c = nc.dram_tensor("b1_c", [F, 1], f32, kind="ExternalInput")
    b2_c = nc.dram_tensor("b2_c", [D, 1], f32, kind="ExternalInput")
    g1_c = nc.dram_tensor("g1_c", [D, 1], f32, kind="ExternalInput")
    bln1_c = nc.dram_tensor("bln1_c", [D, 1], f32, kind="ExternalInput")
    g2_c = nc.dram_tensor("g2_c", [D, 1], f32, kind="ExternalInput")
    bln2_c = nc.dram_tensor("bln2_c", [D, 1], f32, kind="ExternalInput")
    tri2 = nc.dram_tensor("tri2", [128, 2, 128], bf16, kind="ExternalInput")
    outT = nc.dram_tensor("outT", [D, R], f32, kind="ExternalOutput")

    const = ctxpools["const"]
    persist = ctxpools["persist"]
    dram = ctxpools["dram"]

    # ---------------- persistent activations ----------------
    resT = persist.tile([128, DT8, R], f32)    # residual stream (fp32)
    hT1 = persist.tile([128, DT8, R], bf16)    # LN output / attn input
    h2T = persist.tile([128, DT8, R], bf16)    # attn2 out -> FFN in
    OT = persist.tile([128, DT8, R], bf16)     # gathered attention out

    # x first: LN1 is the first consumer
    nc.sync.dma_start(out=resT, in_=xT[:, :].rearrange("(k p) q -> p k q", p=128))

    # ---------------- constants / weights resident in SBUF ----------------
    g1_sb = const.tile([128, DT8], f32)
    nc.sync.dma_start(out=g1_sb, in_=g1_c[:, 0:1].rearrange("(k p) o -> p (k o)", p=128))
    bln1_sb = const.tile([128, DT8], f32)
    nc.sync.dma_start(out=bln1_sb, in_=bln1_c[:, 0:1].rearrange("(k p) o -> p (k o)", p=128))
    g2_sb = const.tile([128, DT8], f32)
    nc.sync.dma_start(out=g2_sb, in_=g2_c[:, 0:1].rearrange("(k p) o -> p (k o)", p=128))
    bln2_sb = const.tile([128, DT8], f32)
    nc.sync.dma_start(out=bln2_sb, in_=bln2_c[:, 0:1].rearrange("(k p) o -> p (k o)", p=128))
    bo_sb = const.tile([128, DT8], f32)
    nc.sync.dma_start(out=bo_sb, in_=bo_c[:, 0:1].rearrange("(k p) o -> p (k o)", p=128))
    b1_sb = const.tile([128, FT32], f32)
    nc.sync.dma_start(out=b1_sb, in_=b1_c[:, 0:1].rearrange("(k p) o -> p (k o)", p=128))
    b2_sb = const.tile([128, DT8], f32)
    nc.sync.dma_start(out=b2_sb, in_=b2_c[:, 0:1].rearrange("(k p) o -> p (k o)", p=128))
    tri2_sb = const.tile([128, 2, 128], bf16)
    nc.sync.dma_start(out=tri2_sb, in_=tri2[:, :, :])
    ones128 = const.tile([128, 128], bf16)
    nc.vector.memset(ones128, 1.0)
    eps_sb = const.tile([128, 1], f32)
    nc.vector.memset(eps_sb, EPS)
    wrm = const.tile([128, R], bf16)
    nc.vector.memset(wrm, 0.25)
    wq_sb = const.tile([128, DT8, 2 * HD], bf16)
    nc.sync.dma_start(out=wq_sb, in_=wq_my[:, :].rearrange("(k p) n -> p k n", p=128))
    wk_sb = const.tile([128, DT8, 2 * HD], bf16)
    nc.sync.dma_start(out=wk_sb, in_=wk_my[:, :].rearrange("(k p) n -> p k n", p=128))
    wv_sb = const.tile([128, DT8, 2 * HD], bf16)
    nc.sync.dma_start(out=wv_sb, in_=wv_my[:, :].rearrange("(k p) n -> p k n", p=128))
    wo_sb = const.tile([128, DT8, D], bf16)
    nc.sync.dma_start(out=wo_sb, in_=wo[:, :].rearrange("(k p) n -> p k n", p=128))

    warm_ctr = [0]

    def warm_pe(n):
        # dead matmuls that keep the HAM clock gate open through a
        # collective window; sunk into a DRAM scratch to survive DCE
        warm_ctr[0] += 1
        with (
            tc.tile_pool(name=f"warm{warm_ctr[0]}", bufs=1) as wp,
            tc.tile_pool(name=f"warm_ps{warm_ctr[0]}", bufs=1,
                         space="PSUM") as wps,
        ):
            psw = wps.tile([128, R], f32)
            for _ in range(n):
                nc.tensor.matmul(psw, ones128, wrm, start=True, stop=True)
            sink = wp.tile([128, R], bf16)
            nc.vector.tensor_copy(out=sink, in_=psw)
            scr = dram.tile([128, R], bf16, tag="warmscr", bufs=2)
            nc.sync.dma_start(out=scr, in_=sink)

    # =================================================================
    def layernorm(g_sb, bln_sb, hT):
        with (
            tc.tile_pool(name="ln_sb", bufs=2) as lnp,
            tc.tile_pool(name="ln_ps", bufs=1, space="PSUM") as lnps,
        ):
            ps_sum = lnps.tile([128, R], f32, tag="s")
            ps_sq = lnps.tile([128, R], f32, tag="q")
            for kt in range(DT8):
                rb = lnp.tile([128, R], bf16, tag="rb", bufs=3)
                nc.vector.tensor_copy(out=rb, in_=resT[:, kt, :])
                sq = lnp.tile([128, R], bf16, tag="sq", bufs=3)
                nc.vector.tensor_mul(out=sq, in0=rb, in1=rb)
                nc.tensor.matmul(ps_sum, ones128, rb,
                                 start=(kt == 0), stop=(kt == DT8 - 1))
                nc.tensor.matmul(ps_sq, ones128, sq,
                                 start=(kt == 0), stop=(kt == DT8 - 1))
            # all per-row scalars live on all 128 partitions (full DVE width)
            mu = lnp.tile([128, R], f32)
            nc.vector.tensor_scalar_mul(out=mu, in0=ps_sum, scalar1=1.0 / D)
            msq = lnp.tile([128, R], f32)
            nc.vector.tensor_scalar_mul(out=msq, in0=ps_sq, scalar1=1.0 / D)
            mu2 = lnp.tile([128, R], f32)
            nc.vector.tensor_mul(out=mu2, in0=mu, in1=mu)
            var = lnp.tile([128, R], f32)
            nc.vector.tensor_tensor(out=var, in0=msq, in1=mu2, op=ALU.subtract)
            sd = lnp.tile([128, R], f32)
            nc.scalar.activation(out=sd, in_=var, func=AF.Sqrt, bias=eps_sb)
            rstd = lnp.tile([128, R], f32)
            nc.vector.reciprocal(out=rstd, in_=sd)
            for kt in range(DT8):
                t1 = lnp.tile([128, R], f32, tag="t1", bufs=3)
                nc.vector.tensor_tensor(out=t1, in0=resT[:, kt, :],
                                        in1=mu, op=ALU.subtract)
                t2 = lnp.tile([128, R], f32, tag="t2", bufs=3)
                nc.vector.tensor_mul(out=t2, in0=t1, in1=rstd)
                nc.vector.tensor_scalar(out=hT[:, kt, :], in0=t2,
                                        scalar1=g_sb[:, kt:kt + 1],
                                        scalar2=bln_sb[:, kt:kt + 1],
                                        op0=ALU.mult, op1=ALU.add)

    # =================================================================
    def attention(hT, first):
        # AllGather the LN output; each core then computes q/k/v for its own
        # 2 heads over the full sequence from host-sliced weight columns.
        ag_in = dram.tile([D, R], bf16, tag="ag1i", bufs=2)
        ag_out = dram.tile([NCORES * D, R], bf16, addr_space="Shared",
                           tag="ag1o", bufs=2)
        nc.sync.dma_start(
            out=ag_in.rearrange("(k p) q -> p k q", p=128), in_=hT)
        nc.gpsimd.collective_compute(
            "AllGather", ALU.bypass, replica_groups=GROUP8,
            ins=[ag_in.opt()], outs=[ag_out.opt()])
        ag1v = ag_out.rearrange("(r k p) q -> r k p q", r=NCORES, k=DT8, p=128)
        warm_pe(110)

        a2a_o_in = dram.tile([NCORES * 128, R], bf16, tag="a2aoi", bufs=2)
        a2a_o_out = dram.tile([NCORES * 128, R], bf16, tag="a2aoo", bufs=2)
        with (
            tc.tile_pool(name="at_sb", bufs=2) as ap,
            tc.tile_pool(name="at_big", bufs=2) as bigp,
        ):
            # ---- phase 1: QKV for my 2 heads, both batches ----
            qTs, kTs, vTs = [], [], []
            with tc.tile_pool(name="qkv_ps", bufs=4, space="PSUM") as qps:
                for b in range(B):
                    hfull = bigp.tile([128, DT8, S], bf16, tag="hfull",
                                      name=f"hfull{b}")
                    for kt in range(DT8):
                        nc.sync.dma_start(
                            out=hfull[:, kt, :].rearrange(
                                "p (r q) -> p r q", r=4),
                            in_=ag1v[4 * b:4 * b + 4, kt].rearrange(
                                "r p q -> p r q"))
                    qT = ap.tile([128, 4, R], bf16, tag="qT", name=f"qT{b}")
                    kT = ap.tile([128, 4, R], bf16, tag="kT", name=f"kT{b}")
                    vT = ap.tile([128, 4, R], bf16, tag="vT", name=f"vT{b}")
                    qTs.append(qT); kTs.append(kT); vTs.append(vT)
                    for dst, wsb, scale in ((kT, wk_sb, None),
                                            (qT, wq_sb, 0.125),
                                            (vT, wv_sb, None)):
                        for c in range(4):
                            ps = qps.tile([128, R], f32, tag="ps")
                            for kt in range(DT8):
                                nc.tensor.matmul(
                                    ps, wsb[:, kt, :],
                                    hfull[:, kt, c * R:(c + 1) * R],
                                    start=(kt == 0), stop=(kt == DT8 - 1))
                            if scale is None:
                                nc.vector.tensor_copy(out=dst[:, c, :], in_=ps)
                            else:
                                nc.vector.tensor_scalar_mul(
                                    out=dst[:, c, :], in0=ps, scalar1=scale)

            # ---- phase 2: V transpose -> v_aug [128(seq), kt, h, 65],
            #      descending kt so the AV loop's first tiles land first ----
            vaugs, sumvs = [], []
            for b in range(B):
                vT = vTs[b]
                vaug = ap.tile([128, KT16, 2, HD + 1], bf16, tag="vaug",
                               name=f"vaug{b}")
                vaugs.append(vaug)
                nc.vector.memset(vaug[:, :, :, HD:HD + 1], 1.0)
                for kt in range(KT16 - 1, -1, -1):
                    vtmp = ap.tile([128, 128], bf16, tag="vtmp", bufs=4)
                    nc.sync.dma_start(
                        out=vtmp,
                        in_=vT[:, kt // 4, (kt % 4) * 128:(kt % 4) * 128 + 128],
                        transpose=True)
                    nc.vector.tensor_copy(out=vaug[:, kt, 0, 0:HD],
                                          in_=vtmp[:, 0:HD])
                    nc.vector.tensor_copy(out=vaug[:, kt, 1, 0:HD],
                                          in_=vtmp[:, HD:128])
                # mean of V over the whole sequence (all-masked last row)
                sumv = ap.tile([128, 1], f32, tag="sumv", name=f"sumv{b}")
                sumvs.append(sumv)
                nc.vector.tensor_reduce(out=sumv, in_=vT,
                                        axis=mybir.AxisListType.XY, op=ALU.add)

            # ---- phase 3: attention cores ----
            for b in range(B):
                qT, kT, vaug, sumv = qTs[b], kTs[b], vaugs[b], sumvs[b]
                oT = ap.tile([128, 4, R], bf16, tag="oT", name=f"oT{b}")
                with (
                    tc.tile_pool(name="sc_ps", bufs=2, space="PSUM") as scps,
                    tc.tile_pool(name="av_ps", bufs=4, space="PSUM") as avps,
                ):
                    for c in range(4):
                        po = [avps.tile([128, R], f32, tag="po", name=f"po{hh}")
                              for hh in range(2)]
                        for kt in range(KT16 - 1, 4 * c - 1, -1):
                            npfx = min(kt - 4 * c + 1, 4) * 128
                            pss = scps.tile([128, 2, R], f32, tag="pss")
                            for hh in range(2):
                                lo = hh * HD
                                nc.tensor.matmul(
                                    pss[:, hh, 0:npfx],
                                    kT[lo:lo + HD, kt // 4,
                                       (kt % 4) * 128:(kt % 4) * 128 + 128],
                                    qT[lo:lo + HD, c, 0:npfx],
                                    start=True, stop=True)
                            wt = ap.tile([128, 2, R], bf16, tag="wt", bufs=3)
                            nc.scalar.activation(out=wt[:, :, 0:npfx],
                                                 in_=pss[:, :, 0:npfx],
                                                 func=AF.Exp)
                            if kt <= 4 * c + 3:
                                # diagonal block: exact multiplicative mask
                                nc.vector.tensor_mul(
                                    out=wt[:, :, npfx - 128:npfx],
                                    in0=wt[:, :, npfx - 128:npfx],
                                    in1=tri2_sb)
                            for hh in range(2):
                                nc.tensor.matmul(
                                    po[hh][0:HD + 1, 0:npfx],
                                    vaug[:, kt, hh, :],
                                    wt[:, hh, 0:npfx],
                                    start=(kt == KT16 - 1),
                                    stop=(kt == 4 * c))
                        # Drain po to SBUF immediately (~1us) so the PSUM
                        # banks recycle; the normalize then runs off the SBUF
                        # copy out-of-band. Denominators (row HD) are
                        # reshaped to [128, 8] through DRAM before inverting
                        # (DVE reciprocal is ~6.4ns/elem along the free dim),
                        # then broadcast.
                        poc = ap.tile([HD + 1, 2, R], f32, tag="poc", bufs=3)
                        for hh in range(2):
                            nc.vector.tensor_copy(out=poc[:, hh, :],
                                                  in_=po[hh][0:HD + 1, :])
                        scr2 = dram.tile([1, 2, R], f32, tag="rscr", bufs=4)
                        nc.sync.dma_start(out=scr2, in_=poc[HD:HD + 1, :, :])
                        denp = ap.tile([128, 8], f32, tag="denp", bufs=4)
                        nc.sync.dma_start(
                            out=denp,
                            in_=scr2[0].rearrange("h (p f) -> (h p) f", p=64))
                        recp = ap.tile([128, 8], f32, tag="recp", bufs=4)
                        nc.vector.reciprocal(out=recp, in_=denp)
                        scr3 = dram.tile([1, 2, R], f32, tag="rscr3", bufs=4)
                        nc.sync.dma_start(
                            out=scr3[0].rearrange("h (p f) -> (h p) f", p=64),
                            in_=recp)
                        recb = ap.tile([HD, 2, R], f32, tag="recb", bufs=2)
                        nc.sync.dma_start(
                            out=recb,
                            in_=scr3[0][None].broadcast_to([HD, 2, R]))
                        for hh in range(2):
                            nc.vector.tensor_mul(
                                out=oT[hh * HD:hh * HD + HD, c, :],
                                in0=poc[0:HD, hh, :], in1=recb[:, hh, :])
                        if c == 3:
                            # all-masked last row: uniform attention = mean(V)
                            nc.vector.tensor_scalar_mul(
                                out=oT[:, 3, R - 1:R], in0=sumv,
                                scalar1=1.0 / S)
                nc.sync.dma_start(
                    out=a2a_o_in[bass.ds(4 * b * 128, 512), :].rearrange(
                        "(c p) q -> p c q", c=4),
                    in_=oT)
        nc.gpsimd.collective_compute(
            "AllToAll", ALU.bypass, replica_groups=GROUP8,
            ins=[a2a_o_in.opt()], outs=[a2a_o_out.opt()])
        warm_pe(70)
        nc.sync.dma_start(
            out=OT, in_=a2a_o_out[:, :].rearrange("(s p) q -> p s q", p=128))

        # ---- out-projection (+ bias, + residual or -> h2T) ----
        with (
            tc.tile_pool(name="op_sb", bufs=3) as opp,
            tc.tile_pool(name="op_ps", bufs=4, space="PSUM") as opps,
        ):
            for nt in range(DT8):
                ps = opps.tile([128, R], f32, tag="ps")
                for s in range(DT8):
                    nc.tensor.matmul(ps, wo_sb[:, s, nt * 128:nt * 128 + 128],
                                     OT[:, s, :],
                                     start=(s == 0), stop=(s == DT8 - 1))
                if first:
                    t = opp.tile([128, R], f32, tag="t")
                    nc.vector.tensor_scalar(out=t, in0=ps,
                                            scalar1=bo_sb[:, nt:nt + 1],
                                            scalar2=None, op0=ALU.add)
                    nc.vector.tensor_add(out=resT[:, nt, :],
                                         in0=resT[:, nt, :], in1=t)
                else:
                    nc.vector.tensor_scalar(out=h2T[:, nt, :], in0=ps,
                                            scalar1=bo_sb[:, nt:nt + 1],
                                            scalar2=None, op0=ALU.add)

    # =================================================================
    # layer body
    layernorm(g1_sb, bln1_sb, hT1)
    attention(hT1, first=True)
    layernorm(g2_sb, bln2_sb, hT1)
    attention(hT1, first=False)

    # ---- FFN ----
    with (
        tc.tile_pool(name="ffn_sb", bufs=3) as fp,
        tc.tile_pool(name="ffn_big", bufs=1) as fbig,
        tc.tile_pool(name="ffn_ps", bufs=4, space="PSUM") as fps,
    ):
        gT = fbig.tile([128, FT32, R], bf16)
        w1v = w1[:, :].rearrange("(k p) (nt n) -> nt k p n", p=128, n=128)
        for nt in range(FT32):
            w1t = fp.tile([128, DT8, 128], bf16, tag="w1t", bufs=4)
            nc.sync.dma_start(out=w1t, in_=w1v[nt].rearrange("k p n -> p k n"))
            ps = fps.tile([128, R], f32, tag="ps1")
            for kt in range(DT8):
                nc.tensor.matmul(ps, w1t[:, kt, :], h2T[:, kt, :],
                                 start=(kt == 0), stop=(kt == DT8 - 1))
            nc.scalar.activation(out=gT[:, nt, :], in_=ps, func=AF.Gelu,
                                 bias=b1_sb[:, nt:nt + 1])
        w2v = w2[:, :].rearrange("(k p) (nt n) -> nt k p n", p=128, n=128)
        for nt in range(DT8):
            w2t = fp.tile([128, FT32, 128], bf16, tag="w2t", bufs=3)
            nc.sync.dma_start(out=w2t, in_=w2v[nt].rearrange("k p n -> p k n"))
            ps2 = fps.tile([128, R], f32, tag="ps2")
            for kt in range(FT32):
                nc.tensor.matmul(ps2, w2t[:, kt, :], gT[:, kt, :],
                                 start=(kt == 0), stop=(kt == FT32 - 1))
            t = fp.tile([128, R], f32, tag="t")
            nc.vector.tensor_scalar(out=t, in0=ps2,
                                    scalar1=b2_sb[:, nt:nt + 1],
                                    scalar2=None, op0=ALU.add)
            ot = fp.tile([128, R], f32, tag="ot")
            nc.vector.tensor_add(out=ot, in0=t, in1=resT[:, nt, :])
            nc.sync.dma_start(out=outT[nt * 128:(nt + 1) * 128, :], in_=ot)


def build():
    nc = bacc.Bacc("TRN2", target_bir_lowering=False, debug=False,
                   num_devices=NCORES)
    with tile.TileContext(nc) as tc:
        with (
            tc.tile_pool(name="const", bufs=1) as const,
            tc.tile_pool(name="persist", bufs=1) as persist,
            tc.tile_pool(name="dram", bufs=1, space="DRAM") as dram,
        ):
            _emit(nc, tc, {"const": const, "persist": persist, "dram": dram})
    nc.compile()
    return nc


_CACHED = {}


def _get_nc():
    if "nc" not in _CACHED:
        _CACHED["nc"] = build()
    return _CACHED["nc"]


def _prep_in_maps(inputs):
    gf = lambda k: np.asarray(inputs[k], np.float32)
    x = gf("x")
    wq, wk, wv = gf("wq"), gf("wk"), gf("wv")
    tobf = lambda a: np.ascontiguousarray(a).astype(ml_dtypes.bfloat16)
    col = lambda a: np.ascontiguousarray(gf(a).reshape(-1, 1))
    tri01 = (np.arange(128)[:, None] > np.arange(128)[None, :]).astype(
        ml_dtypes.bfloat16)
    tri2_np = np.ascontiguousarray(np.stack([tri01, tri01], axis=1))
    shared = dict(wo=tobf(gf("wo")), w1=tobf(gf("w1")), w2=tobf(gf("w2")),
                  bo_c=col("bo"), b1_c=col("b1"), b2_c=col("b2"),
                  g1_c=col("ln1_g"), bln1_c=col("ln1_b"),
                  g2_c=col("ln2_g"), bln2_c=col("ln2_b"), tri2=tri2_np)
    in_maps = []
    for c in range(NCORES):
        b, q = c // 4, c % 4
        m = dict(shared)
        m["xT"] = np.ascontiguousarray(x[b, q * R:(q + 1) * R, :].T)
        m["wq_my"] = tobf(wq[:, 128 * c:128 * (c + 1)])
        m["wk_my"] = tobf(wk[:, 128 * c:128 * (c + 1)])
        m["wv_my"] = tobf(wv[:, 128 * c:128 * (c + 1)])
        in_maps.append(m)
    return in_maps


def run(inputs, **kw):
    nc = _get_nc()
    in_maps = _prep_in_maps(inputs)
    res = bass_utils.run_bass_kernel_spmd(nc, in_maps,
                                          core_ids=list(range(NCORES)), **kw)
    out = np.empty((B, S, D), np.float32)
    for c in range(NCORES):
        b, q = c // 4, c % 4
        out[b, q * R:(q + 1) * R, :] = res.results[c]["outT"].T
    return out, res


def kernel(**inputs):
    out, _ = run(inputs)
    return out


# revision 13
# speedup vs baseline: 1.1673x; 1.1673x over previous
"""Trainium2 Bass kernel for a decoder layer (LN->attn->res, LN->attn->FFN->res).

Sharding (8 cores, SPMD single program):
 - Row-parallel phases (LN / out-proj / FFN / residuals): global rows B*S =
   4096 split 512/core: core c owns batch c//4, seq rows [512*(c%4), +512).
 - Attention head-parallel: core c computes 2 global heads {2c, 2c+1} for
   both batches over the full sequence (per-core column-sliced QKV weights).
 - Collectives: one 8-core AllGather of the LN output per attention (the
   cheap direction: 1 MB in / 7 MB received), and one AllToAll of the
   attention outputs (1 MB, ~8x less wire than the AllGather equivalent)
   feeding a local out-projection.
 - Dummy PE matmuls are streamed during both collective windows to keep the
   HAM clock gate at 2.4 GHz (otherwise the post-collective matmuls run at
   1.2 GHz for ~3.4 us).

Layout: activations feature-on-partition ([D, rows]); host pre-transposes x
and post-transposes the output. Matmuls bf16, accumulation fp32, residual fp32.

Masking: reference masks k <= q (strictly-upper attention). Softmax runs
without max-subtraction (scores bounded); both heads' score tiles share one
2-bank PSUM tile so a single Exp covers them; diagonal 128x128 blocks are
masked multiplicatively AFTER exp with a 0/1 upper-strict mask on the vector
engine; fully-masked tiles are skipped via per-tile column prefixes. Softmax
denominators ride in PSUM row 64 via a ones-column appended to V; they are
reshaped to [128,8] through DRAM before the reciprocal (DVE reciprocal is
~6.4ns/elem along the free dim). The all-masked last row (uniform attention
over all 2048 keys) is patched post-normalize with a DVE reduce of V.

LayerNorm stats use a [128,128] ones stationary so every per-row scalar op
runs on all 128 lanes.
"""

import numpy as np
import ml_dtypes

import concourse.bass as bass
import concourse.bacc as bacc
import concourse.tile as tile
import concourse.mybir as mybir
from concourse import bass_utils

B, S, D, H, F = 2, 2048, 1024, 16, 4096
HD = D // H            # 64
NCORES = 8
R = S // 4             # 512 rows per core
KT16 = S // 128        # 16 seq tiles
DT8 = D // 128         # 8 feature tiles of D
FT32 = F // 128        # 32 feature tiles of F
EPS = 1e-5

f32 = mybir.dt.float32
bf16 = mybir.dt.bfloat16
GROUP8 = [list(range(8))]
AF = mybir.ActivationFunctionType
ALU = mybir.AluOpType


def _emit(nc, tc, ctxpools):
    # ---------------- I/O tensors ----------------
    xT = nc.dram_tensor("xT", [D, R], f32, kind="ExternalInput")
    wq_my = nc.dram_tensor("wq_my", [D, 2 * HD], bf16, kind="ExternalInput")
    wk_my = nc.dram_tensor("wk_my", [D, 2 * HD], bf16, kind="ExternalInput")
    wv_my = nc.dram_tensor("wv_my", [D, 2 * HD], bf16, kind="ExternalInput")
    wo = nc.dram_tensor("wo", [D, D], bf16, kind="ExternalInput")
    w1 = nc.dram_tensor("w1", [D, F], bf16, kind="ExternalInput")
    w2 = nc.dram_tensor("w2", [F, D], bf16, kind="ExternalInput")
    bo_c = nc.dram_tensor("bo_c", [D, 1], f32, kind="ExternalInput")
    b1_c = nc.dram_tensor("b1_c", [F, 1], f32, kind="ExternalInput")
    b2_c = nc.dram_tensor("b2_c", [D, 1], f32, kind="ExternalInput")
    g1_c = nc.dram_tensor("g1_c", [D, 1], f32, kind="ExternalInput")
    bln1_c = nc.dram_tensor("bln1_c", [D, 1], f32, kind="ExternalInput")
    g2_c = nc.dram_tensor("g2_c", [D, 1], f32, kind="ExternalInput")
    bln2_c = nc.dram_tensor("bln2_c", [D, 1], f32, kind="ExternalInput")
    tri2 = nc.dram_tensor("tri2", [128, 2, 128], bf16, kind="ExternalInput")
    outT = nc.dram_tensor("outT", [D, R], f32, kind="ExternalOutput")

    const = ctxpools["const"]
    persist = ctxpools["persist"]
    dram = ctxpools["dram"]

    # ---------------- persistent activations ----------------
    resT = persist.tile([128, DT8, R], f32)    # residual stream (fp32)
    hT1 = persist.tile([128, DT8, R], bf16)    # LN output / attn input
    h2T = persist.tile([128, DT8, R], bf16)    # attn2 out -> FFN in
    OT = persist.tile([128, DT8, R], bf16)     # gathered attention out

    # x first: LN1 is the first consumer
    nc.sync.dma_start(out=resT, in_=xT[:, :].rearrange("(k p) q -> p k q", p=128))

    # ---------------- constants / weights resident in SBUF ----------------
    g1_sb = const.tile([128, DT8], f32)
    nc.sync.dma_start(out=g1_sb, in_=g1_c[:, 0:1].rearrange("(k p) o -> p (k o)", p=128))
    bln1_sb = const.tile([128, DT8], f32)
    nc.sync.dma_start(out=bln1_sb, in_=bln1_c[:, 0:1].rearrange("(k p) o -> p (k o)", p=128))
    g2_sb = const.tile([128, DT8], f32)
    nc.sync.dma_start(out=g2_sb, in_=g2_c[:, 0:1].rearrange("(k p) o -> p (k o)", p=128))
    bln2_sb = const.tile([128, DT8], f32)
    nc.sync.dma_start(out=bln2_sb, in_=bln2_c[:, 0:1].rearrange("(k p) o -> p (k o)", p=128))
    bo_sb = const.tile([128, DT8], f32)
    nc.sync.dma_start(out=bo_sb, in_=bo_c[:, 0:1].rearrange("(k p) o -> p (k o)", p=128))
    b1_sb = const.tile([128, FT32], f32)
    nc.sync.dma_start(out=b1_sb, in_=b1_c[:, 0:1].rearrange("(k p) o -> p (k o)", p=128))
    b2_sb = const.tile([128, DT8], f32)
    nc.sync.dma_start(out=b2_sb, in_=b2_c[:, 0:1].rearrange("(k p) o -> p (k o)", p=128))
    tri2_sb = const.tile([128, 2, 128], bf16)
    nc.sync.dma_start(out=tri2_sb, in_=tri2[:, :, :])
    ones128 = const.tile([128, 128], bf16)
    nc.vector.memset(ones128, 1.0)
    eps_sb = const.tile([128, 1], f32)
    nc.vector.memset(eps_sb, EPS)
    wrm = const.tile([128, R], bf16)
    nc.vector.memset(wrm, 0.25)
    wq_sb = const.tile([128, DT8, 2 * HD], bf16)
    nc.sync.dma_start(out=wq_sb, in_=wq_my[:, :].rearrange("(k p) n -> p k n", p=128))
    wk_sb = const.tile([128, DT8, 2 * HD], bf16)
    nc.sync.dma_start(out=wk_sb, in_=wk_my[:, :].rearrange("(k p) n -> p k n", p=128))
    wv_sb = const.tile([128, DT8, 2 * HD], bf16)
    nc.sync.dma_start(out=wv_sb, in_=wv_my[:, :].rearrange("(k p) n -> p k n", p=128))
    wo_sb = const.tile([128, DT8, D], bf16)
    nc.sync.dma_start(out=wo_sb, in_=wo[:, :].rearrange("(k p) n -> p k n", p=128))

    warm_ctr = [0]

    def warm_pe(n):
        # dead matmuls that keep the HAM clock gate open through a
        # collective window; sunk into a DRAM scratch to survive DCE
        warm_ctr[0] += 1
        with (
            tc.tile_pool(name=f"warm{warm_ctr[0]}", bufs=1) as wp,
            tc.tile_pool(name=f"warm_ps{warm_ctr[0]}", bufs=1,
                         space="PSUM") as wps,
        ):
            psw = wps.tile([128, R], f32)
            for _ in range(n):
                nc.tensor.matmul(psw, ones128, wrm, start=True, stop=True)
            sink = wp.tile([128, R], bf16)
            nc.vector.tensor_copy(out=sink, in_=psw)
            scr = dram.tile([128, R], bf16, tag="warmscr", bufs=2)
            nc.sync.dma_start(out=scr, in_=sink)

    # =================================================================
    def layernorm(g_sb, bln_sb, hT):
        with (
            tc.tile_pool(name="ln_sb", bufs=2) as lnp,
            tc.tile_pool(name="ln_ps", bufs=1, space="PSUM") as lnps,
        ):
            ps_sum = lnps.tile([128, R], f32, tag="s")
            ps_sq = lnps.tile([128, R], f32, tag="q")
            for kt in range(DT8):
                rb = lnp.tile([128, R], bf16, tag="rb", bufs=3)
                nc.vector.tensor_copy(out=rb, in_=resT[:, kt, :])
                sq = lnp.tile([128, R], bf16, tag="sq", bufs=3)
                nc.vector.tensor_mul(out=sq, in0=rb, in1=rb)
                nc.tensor.matmul(ps_sum, ones128, rb,
                                 start=(kt == 0), stop=(kt == DT8 - 1))
                nc.tensor.matmul(ps_sq, ones128, sq,
                                 start=(kt == 0), stop=(kt == DT8 - 1))
            # all per-row scalars live on all 128 partitions (full DVE width)
            mu = lnp.tile([128, R], f32)
            nc.vector.tensor_scalar_mul(out=mu, in0=ps_sum, scalar1=1.0 / D)
            msq = lnp.tile([128, R], f32)
            nc.vector.tensor_scalar_mul(out=msq, in0=ps_sq, scalar1=1.0 / D)
            mu2 = lnp.tile([128, R], f32)
            nc.vector.tensor_mul(out=mu2, in0=mu, in1=mu)
            var = lnp.tile([128, R], f32)
            nc.vector.tensor_tensor(out=var, in0=msq, in1=mu2, op=ALU.subtract)
            sd = lnp.tile([128, R], f32)
            nc.scalar.activation(out=sd, in_=var, func=AF.Sqrt, bias=eps_sb)
            rstd = lnp.tile([128, R], f32)
            nc.vector.reciprocal(out=rstd, in_=sd)
            for kt in range(DT8):
                t1 = lnp.tile([128, R], f32, tag="t1", bufs=3)
                nc.vector.tensor_tensor(out=t1, in0=resT[:, kt, :],
                                        in1=mu, op=ALU.subtract)
                t2 = lnp.tile([128, R], f32, tag="t2", bufs=3)
                nc.vector.tensor_mul(out=t2, in0=t1, in1=rstd)
                nc.vector.tensor_scalar(out=hT[:, kt, :], in0=t2,
                                        scalar1=g_sb[:, kt:kt + 1],
                                        scalar2=bln_sb[:, kt:kt + 1],
                                        op0=ALU.mult, op1=ALU.add)

    # =================================================================
    def attention(hT, first):
        # AllGather the LN output; each core then computes q/k/v for its own
        # 2 heads over the full sequence from host-sliced weight columns.
        ag_in = dram.tile([D, R], bf16, tag="ag1i", bufs=2)
        ag_out = dram.tile([NCORES * D, R], bf16, addr_space="Shared",
                           tag="ag1o", bufs=2)
        nc.sync.dma_start(
            out=ag_in.rearrange("(k p) q -> p k q", p=128), in_=hT)
        nc.gpsimd.collective_compute(
            "AllGather", ALU.bypass, replica_groups=GROUP8,
            ins=[ag_in.opt()], outs=[ag_out.opt()])
        ag1v = ag_out.rearrange("(r k p) q -> r k p q", r=NCORES, k=DT8, p=128)
        warm_pe(110)

        a2a_o_in = dram.tile([NCORES * 128, R], bf16, tag="a2aoi", bufs=2)
        a2a_o_out = dram.tile([NCORES * 128, R], bf16, tag="a2aoo", bufs=2)
        with (
            tc.tile_pool(name="at_sb", bufs=2) as ap,
            tc.tile_pool(name="at_big", bufs=2) as bigp,
        ):
            for b in range(B):
                # ---- gather LN output for batch b: [128, kt, 2048] ----
                hfull = bigp.tile([128, DT8, S], bf16, tag="hfull")
                for kt in range(DT8):
                    nc.sync.dma_start(
                        out=hfull[:, kt, :].rearrange("p (r q) -> p r q", r=4),
                        in_=ag1v[4 * b:4 * b + 4, kt].rearrange(
                            "r p q -> p r q"))

                # ---- QKV for my 2 heads, full sequence ----
                qT = ap.tile([128, 4, R], bf16, tag="qT")
                kT = ap.tile([128, 4, R], bf16, tag="kT")
                vT = ap.tile([128, 4, R], bf16, tag="vT")
                with tc.tile_pool(name="qkv_ps", bufs=4, space="PSUM") as qps:
                    for dst, wsb, scale in ((vT, wv_sb, None),
                                            (kT, wk_sb, None),
                                            (qT, wq_sb, 0.125)):
                        for c in range(4):
                            ps = qps.tile([128, R], f32, tag="ps")
                            for kt in range(DT8):
                                nc.tensor.matmul(
                                    ps, wsb[:, kt, :],
                                    hfull[:, kt, c * R:(c + 1) * R],
                                    start=(kt == 0), stop=(kt == DT8 - 1))
                            if scale is None:
                                nc.vector.tensor_copy(out=dst[:, c, :], in_=ps)
                            else:
                                nc.vector.tensor_scalar_mul(
                                    out=dst[:, c, :], in0=ps, scalar1=scale)

                # ---- V transpose -> v_aug [128(seq), kt, h, 65],
                #      descending kt so the AV loop's first tiles land first
                vaug = ap.tile([128, KT16, 2, HD + 1], bf16, tag="vaug")
                nc.vector.memset(vaug[:, :, :, HD:HD + 1], 1.0)
                for kt in range(KT16 - 1, -1, -1):
                    vtmp = ap.tile([128, 128], bf16, tag="vtmp", bufs=4)
                    nc.sync.dma_start(
                        out=vtmp,
                        in_=vT[:, kt // 4, (kt % 4) * 128:(kt % 4) * 128 + 128],
                        transpose=True)
                    nc.vector.tensor_copy(out=vaug[:, kt, 0, 0:HD],
                                          in_=vtmp[:, 0:HD])
                    nc.vector.tensor_copy(out=vaug[:, kt, 1, 0:HD],
                                          in_=vtmp[:, HD:128])
                # mean of V over the whole sequence (all-masked last row)
                sumv = ap.tile([128, 1], f32, tag="sumv")
                nc.vector.tensor_reduce(out=sumv, in_=vT,
                                        axis=mybir.AxisListType.XY, op=ALU.add)

                oT = ap.tile([128, 4, R], bf16, tag="oT")
                with (
                    tc.tile_pool(name="sc_ps", bufs=2, space="PSUM") as scps,
                    tc.tile_pool(name="av_ps", bufs=4, space="PSUM") as avps,
                ):
                    for c in range(4):
                        po = [avps.tile([128, R], f32, tag="po", name=f"po{hh}")
                              for hh in range(2)]
                        for kt in range(KT16 - 1, 4 * c - 1, -1):
                            npfx = min(kt - 4 * c + 1, 4) * 128
                            pss = scps.tile([128, 2, R], f32, tag="pss")
                            for hh in range(2):
                                lo = hh * HD
                                nc.tensor.matmul(
                                    pss[:, hh, 0:npfx],
                                    kT[lo:lo + HD, kt // 4,
                                       (kt % 4) * 128:(kt % 4) * 128 + 128],
                                    qT[lo:lo + HD, c, 0:npfx],
                                    start=True, stop=True)
                            wt = ap.tile([128, 2, R], bf16, tag="wt", bufs=3)
                            nc.scalar.activation(out=wt[:, :, 0:npfx],
                                                 in_=pss[:, :, 0:npfx],
                                                 func=AF.Exp)
                            if kt <= 4 * c + 3:
                                # diagonal block: exact multiplicative mask
                                nc.vector.tensor_mul(
                                    out=wt[:, :, npfx - 128:npfx],
                                    in0=wt[:, :, npfx - 128:npfx],
                                    in1=tri2_sb)
                            for hh in range(2):
                                nc.tensor.matmul(
                                    po[hh][0:HD + 1, 0:npfx],
                                    vaug[:, kt, hh, :],
                                    wt[:, hh, 0:npfx],
                                    start=(kt == KT16 - 1),
                                    stop=(kt == 4 * c))
                        # Drain po to SBUF immediately (~1us) so the PSUM
                        # banks recycle; the normalize then runs off the SBUF
                        # copy out-of-band. Denominators (row HD) are
                        # reshaped to [128, 8] through DRAM before inverting
                        # (DVE reciprocal is ~6.4ns/elem along the free dim),
                        # then broadcast.
                        poc = ap.tile([HD + 1, 2, R], f32, tag="poc", bufs=3)
                        for hh in range(2):
                            nc.vector.tensor_copy(out=poc[:, hh, :],
                                                  in_=po[hh][0:HD + 1, :])
                        scr2 = dram.tile([1, 2, R], f32, tag="rscr", bufs=4)
                        nc.sync.dma_start(out=scr2, in_=poc[HD:HD + 1, :, :])
                        denp = ap.tile([128, 8], f32, tag="denp", bufs=4)
                        nc.sync.dma_start(
                            out=denp,
                            in_=scr2[0].rearrange("h (p f) -> (h p) f", p=64))
                        recp = ap.tile([128, 8], f32, tag="recp", bufs=4)
                        nc.vector.reciprocal(out=recp, in_=denp)
                        scr3 = dram.tile([1, 2, R], f32, tag="rscr3", bufs=4)
                        nc.sync.dma_start(
                            out=scr3[0].rearrange("h (p f) -> (h p) f", p=64),
                            in_=recp)
                        recb = ap.tile([HD, 2, R], f32, tag="recb", bufs=2)
                        nc.sync.dma_start(
                            out=recb,
                            in_=scr3[0][None].broadcast_to([HD, 2, R]))
                        for hh in range(2):
                            nc.vector.tensor_mul(
                                out=oT[hh * HD:hh * HD + HD, c, :],
                                in0=poc[0:HD, hh, :], in1=recb[:, hh, :])
                        if c == 3:
                            # all-masked last row: uniform attention = mean(V)
                            nc.vector.tensor_scalar_mul(
                                out=oT[:, 3, R - 1:R], in0=sumv,
                                scalar1=1.0 / S)
                nc.sync.dma_start(
                    out=a2a_o_in[bass.ds(4 * b * 128, 512), :].rearrange(
                        "(c p) q -> p c q", c=4),
                    in_=oT)
        nc.gpsimd.collective_compute(
            "AllToAll", ALU.bypass, replica_groups=GROUP8,
            ins=[a2a_o_in.opt()], outs=[a2a_o_out.opt()])
        warm_pe(70)
        nc.sync.dma_start(
            out=OT, in_=a2a_o_out[:, :].rearrange("(s p) q -> p s q", p=128))

        # ---- out-projection (+ bias, + residual or -> h2T) ----
        with (
            tc.tile_pool(name="op_sb", bufs=3) as opp,
            tc.tile_pool(name="op_ps", bufs=4, space="PSUM") as opps,
        ):
            for nt in range(DT8):
                ps = opps.tile([128, R], f32, tag="ps")
                for s in range(DT8):
                    nc.tensor.matmul(ps, wo_sb[:, s, nt * 128:nt * 128 + 128],
                                     OT[:, s, :],
                                     start=(s == 0), stop=(s == DT8 - 1))
                if first:
                    t = opp.tile([128, R], f32, tag="t")
                    nc.vector.tensor_scalar(out=t, in0=ps,
                                            scalar1=bo_sb[:, nt:nt + 1],
                                            scalar2=None, op0=ALU.add)
                    nc.vector.tensor_add(out=resT[:, nt, :],
                                         in0=resT[:, nt, :], in1=t)
                else:
                    nc.vector.tensor_scalar(out=h2T[:, nt, :], in0=ps,
                                            scalar1=bo_sb[:, nt:nt + 1],
                                            scalar2=None, op0=ALU.add)

    # =================================================================
    # layer body
    layernorm(g1_sb, bln1_sb, hT1)
    attention(hT1, first=True)
    layernorm(g2_sb, bln2_sb, hT1)
    attention(hT1, first=False)

    # ---- FFN ----
    with (
        tc.tile_pool(name="ffn_sb", bufs=3) as fp,
        tc.tile_pool(name="ffn_big", bufs=1) as fbig,
        tc.tile_pool(name="ffn_ps", bufs=4, space="PSUM") as fps,
    ):
        gT = fbig.tile([128, FT32, R], bf16)
        w1v = w1[:, :].rearrange("(k p) (nt n) -> nt k p n", p=128, n=128)
        for nt in range(FT32):
            w1t = fp.tile([128, DT8, 128], bf16, tag="w1t", bufs=4)
            nc.sync.dma_start(out=w1t, in_=w1v[nt].rearrange("k p n -> p k n"))
            ps = fps.tile([128, R], f32, tag="ps1")
            for kt in range(DT8):
                nc.tensor.matmul(ps, w1t[:, kt, :], h2T[:, kt, :],
                                 start=(kt == 0), stop=(kt == DT8 - 1))
            nc.scalar.activation(out=gT[:, nt, :], in_=ps, func=AF.Gelu,
                                 bias=b1_sb[:, nt:nt + 1])
        w2v = w2[:, :].rearrange("(k p) (nt n) -> nt k p n", p=128, n=128)
        for nt in range(DT8):
            w2t = fp.tile([128, FT32, 128], bf16, tag="w2t", bufs=3)
            nc.sync.dma_start(out=w2t, in_=w2v[nt].rearrange("k p n -> p k n"))
            ps2 = fps.tile([128, R], f32, tag="ps2")
            for kt in range(FT32):
                nc.tensor.matmul(ps2, w2t[:, kt, :], gT[:, kt, :],
                                 start=(kt == 0), stop=(kt == FT32 - 1))
            t = fp.tile([128, R], f32, tag="t")
            nc.vector.tensor_scalar(out=t, in0=ps2,
                                    scalar1=b2_sb[:, nt:nt + 1],
                                    scalar2=None, op0=ALU.add)
            ot = fp.tile([128, R], f32, tag="ot")
            nc.vector.tensor_add(out=ot, in0=t, in1=resT[:, nt, :])
            nc.sync.dma_start(out=outT[nt * 128:(nt + 1) * 128, :], in_=ot)


def build():
    nc = bacc.Bacc("TRN2", target_bir_lowering=False, debug=False,
                   num_devices=NCORES)
    with tile.TileContext(nc) as tc:
        with (
            tc.tile_pool(name="const", bufs=1) as const,
            tc.tile_pool(name="persist", bufs=1) as persist,
            tc.tile_pool(name="dram", bufs=1, space="DRAM") as dram,
        ):
            _emit(nc, tc, {"const": const, "persist": persist, "dram": dram})
    nc.compile()
    return nc


_CACHED = {}


def _get_nc():
    if "nc" not in _CACHED:
        _CACHED["nc"] = build()
    return _CACHED["nc"]


def _prep_in_maps(inputs):
    gf = lambda k: np.asarray(inputs[k], np.float32)
    x = gf("x")
    wq, wk, wv = gf("wq"), gf("wk"), gf("wv")
    tobf = lambda a: np.ascontiguousarray(a).astype(ml_dtypes.bfloat16)
    col = lambda a: np.ascontiguousarray(gf(a).reshape(-1, 1))
    tri01 = (np.arange(128)[:, None] > np.arange(128)[None, :]).astype(
        ml_dtypes.bfloat16)
    tri2_np = np.ascontiguousarray(np.stack([tri01, tri01], axis=1))
    shared = dict(wo=tobf(gf("wo")), w1=tobf(gf("w1")), w2=tobf(gf("w2")),
                  bo_c=col("bo"), b1_c=col("b1"), b2_c=col("b2"),
                  g1_c=col("ln1_g"), bln1_c=col("ln1_b"),
                  g2_c=col("ln2_g"), bln2_c=col("ln2_b"), tri2=tri2_np)
    in_maps = []
    for c in range(NCORES):
        b, q = c // 4, c % 4
        m = dict(shared)
        m["xT"] = np.ascontiguousarray(x[b, q * R:(q + 1) * R, :].T)
        m["wq_my"] = tobf(wq[:, 128 * c:128 * (c + 1)])
        m["wk_my"] = tobf(wk[:, 128 * c:128 * (c + 1)])
        m["wv_my"] = tobf(wv[:, 128 * c:128 * (c + 1)])
        in_maps.append(m)
    return in_maps


def run(inputs, **kw):
    nc = _get_nc()
    in_maps = _prep_in_maps(inputs)
    res = bass_utils.run_bass_kernel_spmd(nc, in_maps,
                                          core_ids=list(range(NCORES)), **kw)
    out = np.empty((B, S, D), np.float32)
    for c in range(NCORES):
        b, q = c // 4, c % 4
        out[b, q * R:(q + 1) * R, :] = res.results[c]["outT"].T
    return out, res


def kernel(**inputs):
    out, _ = run(inputs)
    return out


# revision 14
# speedup vs baseline: 1.1787x; 1.0098x over previous
"""Trainium2 Bass kernel for a decoder layer (LN->attn->res, LN->attn->FFN->res).

Sharding (8 cores, SPMD single program):
 - Row-parallel phases (LN / out-proj / FFN / residuals): global rows B*S =
   4096 split 512/core: core c owns batch c//4, seq rows [512*(c%4), +512).
 - Attention head-parallel: core c computes 2 global heads {2c, 2c+1} for
   both batches over the full sequence (per-core column-sliced QKV weights).
 - Collectives: one 8-core AllGather of the LN output per attention (the
   cheap direction: 1 MB in / 7 MB received), and one AllToAll of the
   attention outputs (1 MB, ~8x less wire than the AllGather equivalent)
   feeding a local out-projection.
 - Dummy PE matmuls are streamed during both collective windows to keep the
   HAM clock gate at 2.4 GHz (otherwise the post-collective matmuls run at
   1.2 GHz for ~3.4 us).

Layout: activations feature-on-partition ([D, rows]); host pre-transposes x
and post-transposes the output. Matmuls bf16, accumulation fp32, residual fp32.

Masking: reference masks k <= q (strictly-upper attention). Softmax runs
without max-subtraction (scores bounded); both heads' score tiles share one
2-bank PSUM tile so a single Exp covers them; diagonal 128x128 blocks are
masked multiplicatively AFTER exp with a 0/1 upper-strict mask on the vector
engine; fully-masked tiles are skipped via per-tile column prefixes. Softmax
denominators ride in PSUM row 64 via a ones-column appended to V; they are
reshaped to [128,8] through DRAM before the reciprocal (DVE reciprocal is
~6.4ns/elem along the free dim). The all-masked last row (uniform attention
over all 2048 keys) is patched post-normalize with a DVE reduce of V.

LayerNorm stats use a [128,128] ones stationary so every per-row scalar op
runs on all 128 lanes.
"""

import numpy as np
import ml_dtypes

import concourse.bass as bass
import concourse.bacc as bacc
import concourse.tile as tile
import concourse.mybir as mybir
from concourse import bass_utils

B, S, D, H, F = 2, 2048, 1024, 16, 4096
HD = D // H            # 64
NCORES = 8
R = S // 4             # 512 rows per core
KT16 = S // 128        # 16 seq tiles
DT8 = D // 128         # 8 feature tiles of D
FT32 = F // 128        # 32 feature tiles of F
EPS = 1e-5

f32 = mybir.dt.float32
bf16 = mybir.dt.bfloat16
GROUP8 = [list(range(8))]
AF = mybir.ActivationFunctionType
ALU = mybir.AluOpType


def _emit(nc, tc, ctxpools):
    # ---------------- I/O tensors ----------------
    xT = nc.dram_tensor("xT", [D, R], f32, kind="ExternalInput")
    wq_my = nc.dram_tensor("wq_my", [D, 2 * HD], bf16, kind="ExternalInput")
    wk_my = nc.dram_tensor("wk_my", [D, 2 * HD], bf16, kind="ExternalInput")
    wv_my = nc.dram_tensor("wv_my", [D, 2 * HD], bf16, kind="ExternalInput")
    wo = nc.dram_tensor("wo", [D, D], bf16, kind="ExternalInput")
    w1 = nc.dram_tensor("w1", [D, F], bf16, kind="ExternalInput")
    w2 = nc.dram_tensor("w2", [F, D], bf16, kind="ExternalInput")
    bo_c = nc.dram_tensor("bo_c", [D, 1], f32, kind="ExternalInput")
    b1_c = nc.dram_tensor("b1_c", [F, 1], f32, kind="ExternalInput")
    b2_c = nc.dram_tensor("b2_c", [D, 1], f32, kind="ExternalInput")
    g1_c = nc.dram_tensor("g1_c", [D, 1], f32, kind="ExternalInput")
    bln1_c = nc.dram_tensor("bln1_c", [D, 1], f32, kind="ExternalInput")
    g2_c = nc.dram_tensor("g2_c", [D, 1], f32, kind="ExternalInput")
    bln2_c = nc.dram_tensor("bln2_c", [D, 1], f32, kind="ExternalInput")
    tri2 = nc.dram_tensor("tri2", [128, 2, 128], bf16, kind="ExternalInput")
    outT = nc.dram_tensor("outT", [D, R], f32, kind="ExternalOutput")

    const = ctxpools["const"]
    persist = ctxpools["persist"]
    dram = ctxpools["dram"]

    # ---------------- persistent activations ----------------
    resT = persist.tile([128, DT8, R], f32)    # residual stream (fp32)
    hT1 = persist.tile([128, DT8, R], bf16)    # LN output / attn input
    h2T = persist.tile([128, DT8, R], bf16)    # attn2 out -> FFN in
    OT = persist.tile([128, DT8, R], bf16)     # gathered attention out

    # x first: LN1 is the first consumer
    nc.sync.dma_start(out=resT, in_=xT[:, :].rearrange("(k p) q -> p k q", p=128))

    # ---------------- constants / weights resident in SBUF ----------------
    g1_sb = const.tile([128, DT8], f32)
    nc.sync.dma_start(out=g1_sb, in_=g1_c[:, 0:1].rearrange("(k p) o -> p (k o)", p=128))
    bln1_sb = const.tile([128, DT8], f32)
    nc.sync.dma_start(out=bln1_sb, in_=bln1_c[:, 0:1].rearrange("(k p) o -> p (k o)", p=128))
    g2_sb = const.tile([128, DT8], f32)
    nc.sync.dma_start(out=g2_sb, in_=g2_c[:, 0:1].rearrange("(k p) o -> p (k o)", p=128))
    bln2_sb = const.tile([128, DT8], f32)
    nc.sync.dma_start(out=bln2_sb, in_=bln2_c[:, 0:1].rearrange("(k p) o -> p (k o)", p=128))
    bo_sb = const.tile([128, DT8], f32)
    nc.sync.dma_start(out=bo_sb, in_=bo_c[:, 0:1].rearrange("(k p) o -> p (k o)", p=128))
    b1_sb = const.tile([128, FT32], f32)
    nc.sync.dma_start(out=b1_sb, in_=b1_c[:, 0:1].rearrange("(k p) o -> p (k o)", p=128))
    b2_sb = const.tile([128, DT8], f32)
    nc.sync.dma_start(out=b2_sb, in_=b2_c[:, 0:1].rearrange("(k p) o -> p (k o)", p=128))
    tri2_sb = const.tile([128, 2, 128], bf16)
    nc.sync.dma_start(out=tri2_sb, in_=tri2[:, :, :])
    ones128 = const.tile([128, 128], bf16)
    nc.vector.memset(ones128, 1.0)
    eps_sb = const.tile([128, 1], f32)
    nc.vector.memset(eps_sb, EPS)
    wrm = const.tile([128, R], bf16)
    nc.vector.memset(wrm, 0.25)
    wq_sb = const.tile([128, DT8, 2 * HD], bf16)
    nc.sync.dma_start(out=wq_sb, in_=wq_my[:, :].rearrange("(k p) n -> p k n", p=128))
    wk_sb = const.tile([128, DT8, 2 * HD], bf16)
    nc.sync.dma_start(out=wk_sb, in_=wk_my[:, :].rearrange("(k p) n -> p k n", p=128))
    wv_sb = const.tile([128, DT8, 2 * HD], bf16)
    nc.sync.dma_start(out=wv_sb, in_=wv_my[:, :].rearrange("(k p) n -> p k n", p=128))
    wo_sb = const.tile([128, DT8, D], bf16)
    nc.sync.dma_start(out=wo_sb, in_=wo[:, :].rearrange("(k p) n -> p k n", p=128))

    warm_ctr = [0]

    def warm_pe(n):
        # dead matmuls that keep the HAM clock gate open through a
        # collective window; sunk into a DRAM scratch to survive DCE
        warm_ctr[0] += 1
        with (
            tc.tile_pool(name=f"warm{warm_ctr[0]}", bufs=1) as wp,
            tc.tile_pool(name=f"warm_ps{warm_ctr[0]}", bufs=1,
                         space="PSUM") as wps,
        ):
            psw = wps.tile([128, R], f32)
            for _ in range(n):
                nc.tensor.matmul(psw, ones128, wrm, start=True, stop=True)
            sink = wp.tile([128, R], bf16)
            nc.vector.tensor_copy(out=sink, in_=psw)
            scr = dram.tile([128, R], bf16, tag="warmscr", bufs=2)
            nc.sync.dma_start(out=scr, in_=sink)

    # =================================================================
    def layernorm(g_sb, bln_sb, hT):
        with (
            tc.tile_pool(name="ln_sb", bufs=2) as lnp,
            tc.tile_pool(name="ln_ps", bufs=1, space="PSUM") as lnps,
        ):
            ps_sum = lnps.tile([128, R], f32, tag="s")
            ps_sq = lnps.tile([128, R], f32, tag="q")
            for kt in range(DT8):
                rb = lnp.tile([128, R], bf16, tag="rb", bufs=3)
                nc.vector.tensor_copy(out=rb, in_=resT[:, kt, :])
                sq = lnp.tile([128, R], bf16, tag="sq", bufs=3)
                nc.vector.tensor_mul(out=sq, in0=rb, in1=rb)
                nc.tensor.matmul(ps_sum, ones128, rb,
                                 start=(kt == 0), stop=(kt == DT8 - 1))
                nc.tensor.matmul(ps_sq, ones128, sq,
                                 start=(kt == 0), stop=(kt == DT8 - 1))
            # all per-row scalars live on all 128 partitions (full DVE width)
            mu = lnp.tile([128, R], f32)
            nc.vector.tensor_scalar_mul(out=mu, in0=ps_sum, scalar1=1.0 / D)
            msq = lnp.tile([128, R], f32)
            nc.vector.tensor_scalar_mul(out=msq, in0=ps_sq, scalar1=1.0 / D)
            mu2 = lnp.tile([128, R], f32)
            nc.vector.tensor_mul(out=mu2, in0=mu, in1=mu)
            var = lnp.tile([128, R], f32)
            nc.vector.tensor_tensor(out=var, in0=msq, in1=mu2, op=ALU.subtract)
            sd = lnp.tile([128, R], f32)
            nc.scalar.activation(out=sd, in_=var, func=AF.Sqrt, bias=eps_sb)
            rstd = lnp.tile([128, R], f32)
            nc.vector.reciprocal(out=rstd, in_=sd)
            for kt in range(DT8):
                t1 = lnp.tile([128, R], f32, tag="t1", bufs=3)
                nc.vector.tensor_tensor(out=t1, in0=resT[:, kt, :],
                                        in1=mu, op=ALU.subtract)
                t2 = lnp.tile([128, R], f32, tag="t2", bufs=3)
                nc.vector.tensor_mul(out=t2, in0=t1, in1=rstd)
                nc.vector.tensor_scalar(out=hT[:, kt, :], in0=t2,
                                        scalar1=g_sb[:, kt:kt + 1],
                                        scalar2=bln_sb[:, kt:kt + 1],
                                        op0=ALU.mult, op1=ALU.add)

    # =================================================================
    def attention(hT, first):
        # AllGather the LN output; each core then computes q/k/v for its own
        # 2 heads over the full sequence from host-sliced weight columns.
        ag_in = dram.tile([D, R], bf16, tag="ag1i", bufs=2)
        ag_out = dram.tile([NCORES * D, R], bf16, addr_space="Shared",
                           tag="ag1o", bufs=2)
        # partition-major staging: slot rows p*8+k so both the staging and
        # the post-gather readback DMAs are contiguous 8KB-per-partition
        nc.sync.dma_start(
            out=ag_in.rearrange("(p k) q -> p k q", k=DT8), in_=hT)
        nc.gpsimd.collective_compute(
            "AllGather", ALU.bypass, replica_groups=GROUP8,
            ins=[ag_in.opt()], outs=[ag_out.opt()])
        ag1v = ag_out.rearrange("(r p k) q -> r p k q", r=NCORES, p=128, k=DT8)
        warm_pe(110)

        a2a_o_in = dram.tile([NCORES * 128, R], bf16, tag="a2aoi", bufs=2)
        a2a_o_out = dram.tile([NCORES * 128, R], bf16, tag="a2aoo", bufs=2)
        with (
            tc.tile_pool(name="at_sb", bufs=2) as ap,
            tc.tile_pool(name="at_big", bufs=2) as bigp,
        ):
            for b in range(B):
                # ---- gather LN output for batch b: [128, c, kt, 512] ----
                hfull = bigp.tile([128, 4, DT8, R], bf16, tag="hfull")
                for cc in range(4):
                    nc.sync.dma_start(out=hfull[:, cc],
                                      in_=ag1v[4 * b + cc])

                # ---- QKV for my 2 heads, full sequence ----
                qT = ap.tile([128, 4, R], bf16, tag="qT")
                kT = ap.tile([128, 4, R], bf16, tag="kT")
                vT = ap.tile([128, 4, R], bf16, tag="vT")
                with tc.tile_pool(name="qkv_ps", bufs=4, space="PSUM") as qps:
                    for dst, wsb, scale in ((vT, wv_sb, None),
                                            (kT, wk_sb, None),
                                            (qT, wq_sb, 0.125)):
                        for c in range(4):
                            ps = qps.tile([128, R], f32, tag="ps")
                            for kt in range(DT8):
                                nc.tensor.matmul(
                                    ps, wsb[:, kt, :],
                                    hfull[:, c, kt, :],
                                    start=(kt == 0), stop=(kt == DT8 - 1))
                            if scale is None:
                                nc.vector.tensor_copy(out=dst[:, c, :], in_=ps)
                            else:
                                nc.vector.tensor_scalar_mul(
                                    out=dst[:, c, :], in0=ps, scalar1=scale)

                # ---- V transpose -> v_aug [128(seq), kt, h, 65],
                #      descending kt so the AV loop's first tiles land first
                vaug = ap.tile([128, KT16, 2, HD + 1], bf16, tag="vaug")
                nc.vector.memset(vaug[:, :, :, HD:HD + 1], 1.0)
                for kt in range(KT16 - 1, -1, -1):
                    vtmp = ap.tile([128, 128], bf16, tag="vtmp", bufs=4)
                    nc.sync.dma_start(
                        out=vtmp,
                        in_=vT[:, kt // 4, (kt % 4) * 128:(kt % 4) * 128 + 128],
                        transpose=True)
                    nc.vector.tensor_copy(out=vaug[:, kt, 0, 0:HD],
                                          in_=vtmp[:, 0:HD])
                    nc.vector.tensor_copy(out=vaug[:, kt, 1, 0:HD],
                                          in_=vtmp[:, HD:128])
                # mean of V over the whole sequence (all-masked last row)
                sumv = ap.tile([128, 1], f32, tag="sumv")
                nc.vector.tensor_reduce(out=sumv, in_=vT,
                                        axis=mybir.AxisListType.XY, op=ALU.add)

                oT = ap.tile([128, 4, R], bf16, tag="oT")
                with (
                    tc.tile_pool(name="sc_ps", bufs=2, space="PSUM") as scps,
                    tc.tile_pool(name="av_ps", bufs=4, space="PSUM") as avps,
                ):
                    for c in range(4):
                        po = [avps.tile([128, R], f32, tag="po", name=f"po{hh}")
                              for hh in range(2)]
                        for kt in range(KT16 - 1, 4 * c - 1, -1):
                            npfx = min(kt - 4 * c + 1, 4) * 128
                            pss = scps.tile([128, 2, R], f32, tag="pss")
                            for hh in range(2):
                                lo = hh * HD
                                nc.tensor.matmul(
                                    pss[:, hh, 0:npfx],
                                    kT[lo:lo + HD, kt // 4,
                                       (kt % 4) * 128:(kt % 4) * 128 + 128],
                                    qT[lo:lo + HD, c, 0:npfx],
                                    start=True, stop=True)
                            wt = ap.tile([128, 2, R], bf16, tag="wt", bufs=3)
                            nc.scalar.activation(out=wt[:, :, 0:npfx],
                                                 in_=pss[:, :, 0:npfx],
                                                 func=AF.Exp)
                            if kt <= 4 * c + 3:
                                # diagonal block: exact multiplicative mask
                                nc.vector.tensor_mul(
                                    out=wt[:, :, npfx - 128:npfx],
                                    in0=wt[:, :, npfx - 128:npfx],
                                    in1=tri2_sb)
                            for hh in range(2):
                                nc.tensor.matmul(
                                    po[hh][0:HD + 1, 0:npfx],
                                    vaug[:, kt, hh, :],
                                    wt[:, hh, 0:npfx],
                                    start=(kt == KT16 - 1),
                                    stop=(kt == 4 * c))
                        # Drain po to SBUF immediately (~1us) so the PSUM
                        # banks recycle; the normalize then runs off the SBUF
                        # copy out-of-band. Denominators (row HD) are
                        # reshaped to [128, 8] through DRAM before inverting
                        # (DVE reciprocal is ~6.4ns/elem along the free dim),
                        # then broadcast.
                        poc = ap.tile([HD + 1, 2, R], f32, tag="poc", bufs=3)
                        for hh in range(2):
                            nc.vector.tensor_copy(out=poc[:, hh, :],
                                                  in_=po[hh][0:HD + 1, :])
                        scr2 = dram.tile([1, 2, R], f32, tag="rscr", bufs=4)
                        nc.sync.dma_start(out=scr2, in_=poc[HD:HD + 1, :, :])
                        denp = ap.tile([128, 8], f32, tag="denp", bufs=4)
                        nc.sync.dma_start(
                            out=denp,
                            in_=scr2[0].rearrange("h (p f) -> (h p) f", p=64))
                        recp = ap.tile([128, 8], f32, tag="recp", bufs=4)
                        nc.vector.reciprocal(out=recp, in_=denp)
                        scr3 = dram.tile([1, 2, R], f32, tag="rscr3", bufs=4)
                        nc.sync.dma_start(
                            out=scr3[0].rearrange("h (p f) -> (h p) f", p=64),
                            in_=recp)
                        recb = ap.tile([HD, 2, R], f32, tag="recb", bufs=2)
                        nc.sync.dma_start(
                            out=recb,
                            in_=scr3[0][None].broadcast_to([HD, 2, R]))
                        for hh in range(2):
                            nc.vector.tensor_mul(
                                out=oT[hh * HD:hh * HD + HD, c, :],
                                in0=poc[0:HD, hh, :], in1=recb[:, hh, :])
                        if c == 3:
                            # all-masked last row: uniform attention = mean(V)
                            nc.vector.tensor_scalar_mul(
                                out=oT[:, 3, R - 1:R], in0=sumv,
                                scalar1=1.0 / S)
                nc.sync.dma_start(
                    out=a2a_o_in[bass.ds(4 * b * 128, 512), :].rearrange(
                        "(c p) q -> p c q", c=4),
                    in_=oT)
        nc.gpsimd.collective_compute(
            "AllToAll", ALU.bypass, replica_groups=GROUP8,
            ins=[a2a_o_in.opt()], outs=[a2a_o_out.opt()])
        warm_pe(70)
        nc.sync.dma_start(
            out=OT, in_=a2a_o_out[:, :].rearrange("(s p) q -> p s q", p=128))

        # ---- out-projection (+ bias, + residual or -> h2T) ----
        with (
            tc.tile_pool(name="op_sb", bufs=3) as opp,
            tc.tile_pool(name="op_ps", bufs=4, space="PSUM") as opps,
        ):
            for nt in range(DT8):
                ps = opps.tile([128, R], f32, tag="ps")
                for s in range(DT8):
                    nc.tensor.matmul(ps, wo_sb[:, s, nt * 128:nt * 128 + 128],
                                     OT[:, s, :],
                                     start=(s == 0), stop=(s == DT8 - 1))
                if first:
                    t = opp.tile([128, R], f32, tag="t")
                    nc.vector.tensor_scalar(out=t, in0=ps,
                                            scalar1=bo_sb[:, nt:nt + 1],
                                            scalar2=None, op0=ALU.add)
                    nc.vector.tensor_add(out=resT[:, nt, :],
                                         in0=resT[:, nt, :], in1=t)
                else:
                    nc.vector.tensor_scalar(out=h2T[:, nt, :], in0=ps,
                                            scalar1=bo_sb[:, nt:nt + 1],
                                            scalar2=None, op0=ALU.add)

    # =================================================================
    # layer body
    layernorm(g1_sb, bln1_sb, hT1)
    attention(hT1, first=True)
    layernorm(g2_sb, bln2_sb, hT1)
    attention(hT1, first=False)

    # ---- FFN ----
    with (
        tc.tile_pool(name="ffn_sb", bufs=3) as fp,
        tc.tile_pool(name="ffn_big", bufs=1) as fbig,
        tc.tile_pool(name="ffn_ps", bufs=4, space="PSUM") as fps,
    ):
        gT = fbig.tile([128, FT32, R], bf16)
        w1v = w1[:, :].rearrange("(k p) (nt n) -> nt k p n", p=128, n=128)
        for nt in range(FT32):
            w1t = fp.tile([128, DT8, 128], bf16, tag="w1t", bufs=4)
            nc.sync.dma_start(out=w1t, in_=w1v[nt].rearrange("k p n -> p k n"))
            ps = fps.tile([128, R], f32, tag="ps1")
            for kt in range(DT8):
                nc.tensor.matmul(ps, w1t[:, kt, :], h2T[:, kt, :],
                                 start=(kt == 0), stop=(kt == DT8 - 1))
            nc.scalar.activation(out=gT[:, nt, :], in_=ps, func=AF.Gelu,
                                 bias=b1_sb[:, nt:nt + 1])
        w2v = w2[:, :].rearrange("(k p) (nt n) -> nt k p n", p=128, n=128)
        for nt in range(DT8):
            w2t = fp.tile([128, FT32, 128], bf16, tag="w2t", bufs=3)
            nc.sync.dma_start(out=w2t, in_=w2v[nt].rearrange("k p n -> p k n"))
            ps2 = fps.tile([128, R], f32, tag="ps2")
            for kt in range(FT32):
                nc.tensor.matmul(ps2, w2t[:, kt, :], gT[:, kt, :],
                                 start=(kt == 0), stop=(kt == FT32 - 1))
            t = fp.tile([128, R], f32, tag="t")
            nc.vector.tensor_scalar(out=t, in0=ps2,
                                    scalar1=b2_sb[:, nt:nt + 1],
                                    scalar2=None, op0=ALU.add)
            ot = fp.tile([128, R], f32, tag="ot")
            nc.vector.tensor_add(out=ot, in0=t, in1=resT[:, nt, :])
            nc.sync.dma_start(out=outT[nt * 128:(nt + 1) * 128, :], in_=ot)


def build():
    nc = bacc.Bacc("TRN2", target_bir_lowering=False, debug=False,
                   num_devices=NCORES)
    with tile.TileContext(nc) as tc:
        with (
            tc.tile_pool(name="const", bufs=1) as const,
            tc.tile_pool(name="persist", bufs=1) as persist,
            tc.tile_pool(name="dram", bufs=1, space="DRAM") as dram,
        ):
            _emit(nc, tc, {"const": const, "persist": persist, "dram": dram})
    nc.compile()
    return nc


_CACHED = {}


def _get_nc():
    if "nc" not in _CACHED:
        _CACHED["nc"] = build()
    return _CACHED["nc"]


def _prep_in_maps(inputs):
    gf = lambda k: np.asarray(inputs[k], np.float32)
    x = gf("x")
    wq, wk, wv = gf("wq"), gf("wk"), gf("wv")
    tobf = lambda a: np.ascontiguousarray(a).astype(ml_dtypes.bfloat16)
    col = lambda a: np.ascontiguousarray(gf(a).reshape(-1, 1))
    tri01 = (np.arange(128)[:, None] > np.arange(128)[None, :]).astype(
        ml_dtypes.bfloat16)
    tri2_np = np.ascontiguousarray(np.stack([tri01, tri01], axis=1))
    shared = dict(wo=tobf(gf("wo")), w1=tobf(gf("w1")), w2=tobf(gf("w2")),
                  bo_c=col("bo"), b1_c=col("b1"), b2_c=col("b2"),
                  g1_c=col("ln1_g"), bln1_c=col("ln1_b"),
                  g2_c=col("ln2_g"), bln2_c=col("ln2_b"), tri2=tri2_np)
    in_maps = []
    for c in range(NCORES):
        b, q = c // 4, c % 4
        m = dict(shared)
        m["xT"] = np.ascontiguousarray(x[b, q * R:(q + 1) * R, :].T)
        m["wq_my"] = tobf(wq[:, 128 * c:128 * (c + 1)])
        m["wk_my"] = tobf(wk[:, 128 * c:128 * (c + 1)])
        m["wv_my"] = tobf(wv[:, 128 * c:128 * (c + 1)])
        in_maps.append(m)
    return in_maps


def run(inputs, **kw):
    nc = _get_nc()
    in_maps = _prep_in_maps(inputs)
    res = bass_utils.run_bass_kernel_spmd(nc, in_maps,
                                          core_ids=list(range(NCORES)), **kw)
    out = np.empty((B, S, D), np.float32)
    for c in range(NCORES):
        b, q = c // 4, c % 4
        out[b, q * R:(q + 1) * R, :] = res.results[c]["outT"].T
    return out, res


def kernel(**inputs):
    out, _ = run(inputs)
    return out
